# revision 1
# baseline (speedup 1.0000x reference)
"""Trainium2 Bass kernel for y = 2*(einsum('bct,oc->bot', pre, W_pre) + b_pre).

Shapes (hardcoded): pre [16, 512, 4096] f32, W_pre [512, 512] f32, b_pre [512] f32.
Sharding: data-parallel over B across 8 cores (2 batches per core).

Per core: out[b, o, t] = 2*(sum_c W[o,c]*pre[b,c,t] + bias[o]) for 2 batches.
PE matmul computes lhsT.T @ rhs with lhsT = W.T tiles [K=128, M=128] and
rhs = pre tiles [K=128, N=512]; accumulate 4 K-tiles into one PSUM bank,
then ScalarE applies out = 2*psum + 2*bias on eviction PSUM->SBUF.
"""

import os
import sys

for _p in ("/opt/trn_rl_repo", "/root/.axon_site/_ro/trn_rl_repo"):
    if os.path.isdir(_p) and _p not in sys.path:
        sys.path.append(_p)

from contextlib import ExitStack

import numpy as np

import concourse.bass as bass
import concourse.tile as tile
from concourse import bacc, mybir
from concourse.bass_utils import run_bass_kernel_spmd

B, C, T = 16, 512, 4096  # batch, channels (in == out), sequence
NCORES = 8
BPC = B // NCORES  # batches per core
P = 128
KT = C // P  # contraction tiles
MT = C // P  # output-channel tiles
NCHUNK = 512  # matmul moving-operand free dim (max for 4-byte dtypes)
NCH = T // NCHUNK
# Input DMA column chunks: small first chunks so the first matmul group's
# data lands early, bigger later ones to amortize DMA issue overhead.
XCS = [512, 512, 1024, 2048]
# Output store groups (in NCHUNK units) per batch: taper the last batch so the
# final DMAs after the last matmul are small.
OGS = {0: [4, 4], 1: [4, 2, 1, 1]}

# float32: exact, 4 cycles/row on PE. float32r (tf32): 1 cycle/row at N>=256.
MM_DTYPE = mybir.dt.float32r

LAST_RESULT = None  # BassKernelResults of the most recent run (for test harness)
_cache = {}


def _build(mm_dtype):
    # Bacc (not plain Bass): its finalize() runs move_matmul_waits_to_ldweights +
    # generate_event_semaphores, which walrus needs — an fp32 self-loading
    # matmul's implicit LDWEIGHTS tolerates only one semaphore wait.
    nc = bacc.Bacc("TRN2", target_bir_lowering=False, debug=False, num_devices=NCORES)
    # When running tf32 matmuls, the BIR verifier requires matmul inputs to be
    # produced as float32r; declaring the DRAM side as float32r (with the host
    # pre-rounding the payload to tf32) satisfies it without a device-side pass.
    in_dt = mm_dtype if mm_dtype == mybir.dt.float32r else mybir.dt.float32
    pre = nc.dram_tensor("pre", [BPC, C, T], in_dt, kind="ExternalInput").ap()
    wt = nc.dram_tensor("wt", [C, C], in_dt, kind="ExternalInput").ap()
    b2 = nc.dram_tensor("b2", [P, MT], mybir.dt.float32, kind="ExternalInput").ap()
    out = nc.dram_tensor("out", [BPC, C, T], mybir.dt.float32, kind="ExternalOutput").ap()

    with ExitStack() as ctx:
        tc = ctx.enter_context(tile.TileContext(nc))
        wpool = ctx.enter_context(tc.tile_pool(name="w", bufs=1))
        bpool = ctx.enter_context(tc.tile_pool(name="bias", bufs=1))
        xpool = ctx.enter_context(tc.tile_pool(name="x", bufs=2))
        opool = ctx.enter_context(tc.tile_pool(name="o", bufs=8))
        pspool = ctx.enter_context(tc.tile_pool(name="ps", bufs=8, space="PSUM"))

        # DMA issue order is consumption order: the first matmul group (batch 0,
        # nch 0, mt 0) needs x chunk 0 (4x256KB) + w column 0 (4x64KB) — those
        # eight transfers fill the eight HW queues' first round.
        xtiles_b0 = [[None] * KT for _ in range(len(XCS))]
        for kt in range(KT):
            x = xpool.tile([P, XCS[0]], in_dt, name=f"x_0_0_{kt}", tag=f"x0_{kt}")
            nc.sync.dma_start(x[:], pre[0, kt * P : (kt + 1) * P, 0 : XCS[0]])
            xtiles_b0[0][kt] = x

        # W.T resident in SBUF as 16 [128, 128] tiles; mt=0 column first.
        wtiles = [[None] * MT for _ in range(KT)]
        for mt in range(MT):
            for kt in range(KT):
                w = wpool.tile([P, P], in_dt, name=f"w_{kt}_{mt}", tag=f"w{kt}{mt}")
                nc.sync.dma_start(
                    w[:], wt[kt * P : (kt + 1) * P, mt * P : (mt + 1) * P]
                )
                wtiles[kt][mt] = w

        btile = bpool.tile([P, MT], mybir.dt.float32)
        nc.sync.dma_start(btile[:], b2[:])

        # nch -> (x tile index, column offset inside that tile)
        xmap = []
        off = 0
        for xi, xcols in enumerate(XCS):
            for o in range(0, xcols, NCHUNK):
                xmap.append((xi, o))
            off += xcols
        assert len(xmap) == NCH

        for b in range(BPC):
            xtiles = [[None] * KT for _ in range(len(XCS))]
            off = 0
            for xi, xcols in enumerate(XCS):
                if b == 0 and xi == 0:
                    xtiles[0] = xtiles_b0[0]
                    off += xcols
                    continue
                for kt in range(KT):
                    # Big trailing chunk single-buffered to stay inside SBUF;
                    # its reload for batch 1 overlaps batch 0's tail compute.
                    x = xpool.tile(
                        [P, xcols], in_dt, name=f"x_{b}_{xi}_{kt}",
                        tag=f"x{xi}_{kt}", bufs=(1 if xi == len(XCS) - 1 else 2),
                    )
                    nc.sync.dma_start(
                        x[:], pre[b, kt * P : (kt + 1) * P, bass.ds(off, xcols)]
                    )
                    xtiles[xi][kt] = x
                off += xcols

            nch = 0
            for og, osize in enumerate(OGS[b]):
                ocols = osize * NCHUNK
                otiles = [
                    opool.tile(
                        [P, ocols], mybir.dt.float32,
                        name=f"o_{b}_{og}_{mt}", tag="o",
                    )
                    for mt in range(MT)
                ]
                obase = nch * NCHUNK
                for j in range(osize):
                    xi, xoff = xmap[nch]
                    for mt in range(MT):
                        ps = pspool.tile([P, NCHUNK], mybir.dt.float32, tag="ps")
                        for kt in range(KT):
                            lhsT = wtiles[kt][mt][:]
                            rhs = xtiles[xi][kt][:, xoff : xoff + NCHUNK]
                            if mm_dtype != in_dt:
                                lhsT = lhsT.bitcast(mm_dtype)
                                rhs = rhs.bitcast(mm_dtype)
                            nc.tensor.matmul(
                                ps[:], lhsT, rhs, start=(kt == 0), stop=(kt == KT - 1)
                            )
                        # W is pre-scaled by 2 on the host, so only + 2*bias
                        # remains; alternate DVE/ACT so neither engine binds.
                        dst = otiles[mt][:, bass.ts(j, NCHUNK)]
                        bias_col = btile[:, mt : mt + 1]
                        if mt % 2 == 0:
                            nc.vector.tensor_scalar_add(dst, ps[:], bias_col)
                        else:
                            nc.scalar.activation(
                                dst,
                                ps[:],
                                mybir.ActivationFunctionType.Identity,
                                bias=bias_col,
                            )
                    nch += 1
                for mt in range(MT):
                    nc.gpsimd.dma_start(
                        out[b, mt * P : (mt + 1) * P, bass.ds(obase, ocols)],
                        otiles[mt][:],
                    )
    # The axon/PJRT exec path serializes nc as-is; finalize here so Bacc's
    # compile passes (register alloc, event-semaphore wait splitting) run.
    nc.finalize()
    return nc


def _round_tf32(a):
    """Round fp32 array to tf32 (10-bit mantissa), round-to-nearest-even."""
    u = a.view(np.uint32)
    r = u + (0xFFF + ((u >> 13) & 1))
    r &= np.uint32(0xFFFFE000)
    # NaN/Inf payloads must not be touched by the carry into the exponent
    special = (u & np.uint32(0x7F800000)) == np.uint32(0x7F800000)
    r[special] = u[special] & np.uint32(0xFFFFE000)
    return r.view(np.float32)


def kernel(pre, W_pre, b_pre):
    global LAST_RESULT
    pre = np.ascontiguousarray(pre, dtype=np.float32)
    # Fold the reference's final y+y into the weights/bias: out = (2W)x + 2b.
    wT = np.ascontiguousarray(np.asarray(W_pre, dtype=np.float32).T * 2.0)
    if MM_DTYPE == mybir.dt.float32r:
        pre = _round_tf32(pre)
        wT = _round_tf32(wT)
    b2 = np.ascontiguousarray(
        (2.0 * np.asarray(b_pre, dtype=np.float32)).reshape(MT, P).T
    )
    key = str(MM_DTYPE)
    if key not in _cache:
        _cache[key] = _build(MM_DTYPE)
    nc = _cache[key]
    in_maps = [
        {"pre": pre[i * BPC : (i + 1) * BPC], "wt": wT, "b2": b2}
        for i in range(NCORES)
    ]
    res = run_bass_kernel_spmd(nc, in_maps, list(range(NCORES)))
    LAST_RESULT = res
    return np.ascontiguousarray(
        np.concatenate([res.results[i]["out"] for i in range(NCORES)], axis=0),
        dtype=np.float32,
    )



# revision 2
# speedup vs baseline: 1.4312x; 1.4312x over previous
"""Trainium2 Bass kernel for y = 2*(einsum('bct,oc->bot', pre, W_pre) + b_pre).

Shapes (hardcoded): pre [16, 512, 4096] f32, W_pre [512, 512] f32, b_pre [512] f32.
Sharding: data-parallel over B across 8 cores (2 batches per core).

Per core: out[b, o, t] = 2*(sum_c W[o,c]*pre[b,c,t] + bias[o]) for 2 batches.
PE matmul computes lhsT.T @ rhs with lhsT = W.T tiles [K=128, M=128] and
rhs = pre tiles [K=128, N=512]; accumulate 4 K-tiles into one PSUM bank,
then ScalarE/DVE apply out = psum + 2*bias on eviction PSUM->SBUF.

All device I/O is fp16 (host casts): the fp32 baseline was HBM-bound
(33.6MB/core at ~358GB/s ~ 94us); fp16 halves traffic to 16.8MB (~47us),
making the kernel PE-bound (~55us of bf16/fp16-rate matmul).
Accuracy: fp16 in/out gives rel err ~5e-4 vs the 2e-2 gate (verified on
the exact dataset offline); fp8 paths exceed the gate (4e-2) so the 2x
DoubleRow PE rate is not usable.
"""

import os
import sys

for _p in ("/opt/trn_rl_repo", "/root/.axon_site/_ro/trn_rl_repo"):
    if os.path.isdir(_p) and _p not in sys.path:
        sys.path.append(_p)

from contextlib import ExitStack

import numpy as np

import concourse.bass as bass
import concourse.tile as tile
from concourse import bacc, mybir
from concourse.bass_utils import run_bass_kernel_spmd

B, C, T = 16, 512, 4096  # batch, channels (in == out), sequence
NCORES = 8
BPC = B // NCORES  # batches per core
P = 128
KT = C // P  # contraction tiles
MT = C // P  # output-channel tiles
NCHUNK = 512  # matmul moving-operand free dim
NCH = T // NCHUNK
# Input DMA column chunks: small first chunks so the first matmul group's
# data lands early, bigger later ones to amortize DMA issue overhead.
XCS = [512, 512, 1024, 2048]
# Output store groups (in NCHUNK units) per batch: taper the last batch so the
# final DMAs after the last matmul are small.
OGS = {0: [4, 4], 1: [4, 2, 1, 1]}

IN_DT = mybir.dt.float16
OUT_DT = mybir.dt.float16

LAST_RESULT = None  # BassKernelResults of the most recent run (for test harness)
_cache = {}


def _build():
    # Bacc (not plain Bass): its finalize() runs move_matmul_waits_to_ldweights +
    # generate_event_semaphores, which walrus needs.
    nc = bacc.Bacc("TRN2", target_bir_lowering=False, debug=False, num_devices=NCORES)
    pre = nc.dram_tensor("pre", [BPC, C, T], IN_DT, kind="ExternalInput").ap()
    wt = nc.dram_tensor("wt", [C, C], IN_DT, kind="ExternalInput").ap()
    b2 = nc.dram_tensor("b2", [P, MT], mybir.dt.float32, kind="ExternalInput").ap()
    out = nc.dram_tensor("out", [BPC, C, T], OUT_DT, kind="ExternalOutput").ap()

    with ExitStack() as ctx:
        tc = ctx.enter_context(tile.TileContext(nc))
        wpool = ctx.enter_context(tc.tile_pool(name="w", bufs=1))
        bpool = ctx.enter_context(tc.tile_pool(name="bias", bufs=1))
        xpool = ctx.enter_context(tc.tile_pool(name="x", bufs=2))
        opool = ctx.enter_context(tc.tile_pool(name="o", bufs=8))
        pspool = ctx.enter_context(tc.tile_pool(name="ps", bufs=8, space="PSUM"))

        # DMA issue order is consumption order: the first matmul group (batch 0,
        # nch 0, mt 0) needs x chunk 0 + w column 0 first.
        xtiles_b0 = [[None] * KT for _ in range(len(XCS))]
        for kt in range(KT):
            x = xpool.tile([P, XCS[0]], IN_DT, name=f"x_0_0_{kt}", tag=f"x0_{kt}")
            nc.sync.dma_start(x[:], pre[0, kt * P : (kt + 1) * P, 0 : XCS[0]])
            xtiles_b0[0][kt] = x

        # W.T resident in SBUF as 16 [128, 128] tiles; mt=0 column first.
        wtiles = [[None] * MT for _ in range(KT)]
        for mt in range(MT):
            for kt in range(KT):
                w = wpool.tile([P, P], IN_DT, name=f"w_{kt}_{mt}", tag=f"w{kt}{mt}")
                nc.sync.dma_start(
                    w[:], wt[kt * P : (kt + 1) * P, mt * P : (mt + 1) * P]
                )
                wtiles[kt][mt] = w

        btile = bpool.tile([P, MT], mybir.dt.float32)
        nc.sync.dma_start(btile[:], b2[:])

        # nch -> (x tile index, column offset inside that tile)
        xmap = []
        off = 0
        for xi, xcols in enumerate(XCS):
            for o in range(0, xcols, NCHUNK):
                xmap.append((xi, o))
            off += xcols
        assert len(xmap) == NCH

        for b in range(BPC):
            xtiles = [[None] * KT for _ in range(len(XCS))]
            off = 0
            for xi, xcols in enumerate(XCS):
                if b == 0 and xi == 0:
                    xtiles[0] = xtiles_b0[0]
                    off += xcols
                    continue
                for kt in range(KT):
                    # Big trailing chunk single-buffered to stay inside SBUF;
                    # its reload for batch 1 overlaps batch 0's tail compute.
                    x = xpool.tile(
                        [P, xcols], IN_DT, name=f"x_{b}_{xi}_{kt}",
                        tag=f"x{xi}_{kt}", bufs=(1 if xi == len(XCS) - 1 else 2),
                    )
                    nc.sync.dma_start(
                        x[:], pre[b, kt * P : (kt + 1) * P, bass.ds(off, xcols)]
                    )
                    xtiles[xi][kt] = x
                off += xcols

            nch = 0
            for og, osize in enumerate(OGS[b]):
                ocols = osize * NCHUNK
                otiles = [
                    opool.tile(
                        [P, ocols], OUT_DT,
                        name=f"o_{b}_{og}_{mt}", tag="o",
                    )
                    for mt in range(MT)
                ]
                obase = nch * NCHUNK
                for j in range(osize):
                    xi, xoff = xmap[nch]
                    for mt in range(MT):
                        ps = pspool.tile([P, NCHUNK], mybir.dt.float32, tag="ps")
                        for kt in range(KT):
                            nc.tensor.matmul(
                                ps[:],
                                wtiles[kt][mt][:],
                                xtiles[xi][kt][:, xoff : xoff + NCHUNK],
                                start=(kt == 0),
                                stop=(kt == KT - 1),
                            )
                        # W is pre-scaled by 2 on the host, so only + 2*bias
                        # remains; alternate DVE/ACT so neither engine binds.
                        dst = otiles[mt][:, bass.ts(j, NCHUNK)]
                        bias_col = btile[:, mt : mt + 1]
                        if mt % 2 == 0:
                            nc.vector.tensor_scalar_add(dst, ps[:], bias_col)
                        else:
                            nc.scalar.activation(
                                dst,
                                ps[:],
                                mybir.ActivationFunctionType.Identity,
                                bias=bias_col,
                            )
                    nch += 1
                for mt in range(MT):
                    nc.gpsimd.dma_start(
                        out[b, mt * P : (mt + 1) * P, bass.ds(obase, ocols)],
                        otiles[mt][:],
                    )
    # The axon/PJRT exec path serializes nc as-is; finalize here so Bacc's
    # compile passes (register alloc, event-semaphore wait splitting) run.
    nc.finalize()
    return nc


def kernel(pre, W_pre, b_pre):
    global LAST_RESULT
    preh = np.ascontiguousarray(np.asarray(pre, dtype=np.float32).astype(np.float16))
    # Fold the reference's final y+y into the weights/bias: out = (2W)x + 2b.
    wT = np.ascontiguousarray(
        (np.asarray(W_pre, dtype=np.float32) * 2.0).T.astype(np.float16)
    )
    b2 = np.ascontiguousarray(
        (2.0 * np.asarray(b_pre, dtype=np.float32)).reshape(MT, P).T
    )
    if "nc" not in _cache:
        _cache["nc"] = _build()
    nc = _cache["nc"]
    in_maps = [
        {"pre": preh[i * BPC : (i + 1) * BPC], "wt": wT, "b2": b2}
        for i in range(NCORES)
    ]
    res = run_bass_kernel_spmd(nc, in_maps, list(range(NCORES)))
    LAST_RESULT = res
    return np.ascontiguousarray(
        np.concatenate([res.results[i]["out"] for i in range(NCORES)], axis=0),
        dtype=np.float32,
    )


# revision 4
# speedup vs baseline: 1.5919x; 1.1123x over previous
"""Trainium2 Bass kernel for y = 2*(einsum('bct,oc->bot', pre, W_pre) + b_pre).

Shapes (hardcoded): pre [16, 512, 4096] f32, W_pre [512, 512] f32, b_pre [512] f32.
Sharding: data-parallel over B across 8 cores (2 batches per core).

Per core: out[b, o, t] = 2*(sum_c W[o,c]*pre[b,c,t] + bias[o]) for 2 batches.
PE matmul computes lhsT.T @ rhs with lhsT = W.T tiles [K=128, M=128] and
rhs = pre tiles [K=128, N=512]; accumulate 4 K-tiles into one PSUM bank,
then ScalarE/DVE apply out = psum + 2*bias on eviction PSUM->SBUF.

All device I/O is fp16 (host casts): the fp32 baseline was HBM-bound
(33.6MB/core at ~358GB/s ~ 94us); fp16 halves traffic to 16.8MB (~47us),
making the kernel PE-bound (~55us of fp16-rate matmul). fp8 would double
the PE rate (DoubleRow) but exceeds the 2e-2 error gate (4e-2 measured
offline on the exact dataset), so fp16 is the precision floor.

DMA instruction count is minimized (HWDGE issue costs ~0.6us each on the
issuing sequencer, serializing the prologue): W is one host-pre-tiled
DMA, each x column-chunk is one DMA covering all 4 K-tiles (transposed
AP), outputs are 12 DMAs. x loads issue on SP (sync), W/bias on ACT
(scalar), outputs on GpSimd (SWDGE) so the three streams don't serialize.
"""

import os
import sys

for _p in ("/opt/trn_rl_repo", "/root/.axon_site/_ro/trn_rl_repo"):
    if os.path.isdir(_p) and _p not in sys.path:
        sys.path.append(_p)

from contextlib import ExitStack

import numpy as np

import concourse.bass as bass
import concourse.tile as tile
from concourse import bacc, mybir
from concourse.bass_utils import run_bass_kernel_spmd

B, C, T = 16, 512, 4096  # batch, channels (in == out), sequence
NCORES = 8
BPC = B // NCORES  # batches per core
P = 128
KT = C // P  # contraction tiles
MT = C // P  # output-channel tiles
NCHUNK = 512  # matmul moving-operand free dim
NCH = T // NCHUNK
# Input DMA column chunks: small first chunk so the first matmul group's
# data lands early, bigger later ones to amortize DMA issue overhead.
XCS = [512, 512, 1024, 2048]
# Output store groups (in NCHUNK units) per batch: taper batch 1 so the
# final DMAs after the last matmul are small.
OGS = {0: [8], 1: [4, 3, 1]}

IN_DT = mybir.dt.float16
OUT_DT = mybir.dt.float16

LAST_RESULT = None  # BassKernelResults of the most recent run (for test harness)
_cache = {}


def _build():
    # Bacc (not plain Bass): its finalize() runs move_matmul_waits_to_ldweights +
    # generate_event_semaphores, which walrus needs.
    nc = bacc.Bacc("TRN2", target_bir_lowering=False, debug=False, num_devices=NCORES)
    # pre viewed as [b, kt, p, t] (same layout as [b, c, t] with c = kt*128+p).
    pre = nc.dram_tensor("pre", [BPC, KT, P, T], IN_DT, kind="ExternalInput").ap()
    # W pre-tiled on host: wt[p, kt*MT+mt, m] = 2*W[mt*128+m, kt*128+p].
    wt = nc.dram_tensor("wt", [P, KT * MT, P], IN_DT, kind="ExternalInput").ap()
    b2 = nc.dram_tensor("b2", [P, MT], mybir.dt.float32, kind="ExternalInput").ap()
    out = nc.dram_tensor("out", [BPC, C, T], OUT_DT, kind="ExternalOutput").ap()

    with ExitStack() as ctx:
        tc = ctx.enter_context(tile.TileContext(nc))
        wpool = ctx.enter_context(tc.tile_pool(name="w", bufs=1))
        bpool = ctx.enter_context(tc.tile_pool(name="bias", bufs=1))
        xpool = ctx.enter_context(tc.tile_pool(name="x", bufs=2))
        opool = ctx.enter_context(tc.tile_pool(name="o", bufs=8))
        pspool = ctx.enter_context(tc.tile_pool(name="ps", bufs=8, space="PSUM"))

        # One DMA per (batch, column chunk), covering all 4 K-tiles: SBUF
        # tile [128, KT, cols] <- dram [kt, p, cols] transposed to [p, kt, cols].
        # Issue order is consumption order; b0 chunk 0 first.
        def load_x(b, xi, off, cols):
            x = xpool.tile([P, KT, cols], IN_DT, name=f"x_{b}_{xi}", tag=f"x{xi}")
            nc.sync.dma_start(
                x[:], pre[b, :, :, bass.ds(off, cols)].transpose([1, 0, 2])
            )
            return x

        xtiles = {}
        xtiles[(0, 0)] = load_x(0, 0, 0, XCS[0])

        # Whole W in one DMA on the other HWDGE engine (ACT): 512KB.
        wtile = wpool.tile([P, KT * MT * P], IN_DT, name="w")
        nc.scalar.dma_start(wtile[:], wt[:].flatten_outer_dims())
        btile = bpool.tile([P, MT], mybir.dt.float32)
        nc.scalar.dma_start(btile[:], b2[:])

        def wslice(kt, mt):
            return wtile[:, (kt * MT + mt) * P : (kt * MT + mt + 1) * P]

        off = XCS[0]
        for xi in range(1, len(XCS)):
            xtiles[(0, xi)] = load_x(0, xi, off, XCS[xi])
            off += XCS[xi]
        off = 0
        for xi in range(len(XCS)):
            xtiles[(1, xi)] = load_x(1, xi, off, XCS[xi])
            off += XCS[xi]

        # nch -> (x tile index, column offset inside that tile)
        xmap = []
        off = 0
        for xi, xcols in enumerate(XCS):
            for o in range(0, xcols, NCHUNK):
                xmap.append((xi, o))
            off += xcols
        assert len(xmap) == NCH

        for b in range(BPC):
            nch = 0
            for og, osize in enumerate(OGS[b]):
                ocols = osize * NCHUNK
                otiles = [
                    opool.tile([P, ocols], OUT_DT, name=f"o_{b}_{og}_{mt}", tag="o")
                    for mt in range(MT)
                ]
                obase = nch * NCHUNK
                for j in range(osize):
                    xi, xoff = xmap[nch]
                    for mt in range(MT):
                        ps = pspool.tile([P, NCHUNK], mybir.dt.float32, tag="ps")
                        for kt in range(KT):
                            nc.tensor.matmul(
                                ps[:],
                                wslice(kt, mt),
                                xtiles[(b, xi)][:, kt, xoff : xoff + NCHUNK],
                                start=(kt == 0),
                                stop=(kt == KT - 1),
                            )
                        # W is pre-scaled by 2 on the host, so only + 2*bias
                        # remains; alternate DVE/ACT so neither engine binds.
                        dst = otiles[mt][:, bass.ts(j, NCHUNK)]
                        bias_col = btile[:, mt : mt + 1]
                        if (j + mt) % 2 == 0:
                            nc.vector.tensor_scalar_add(dst, ps[:], bias_col)
                        else:
                            nc.scalar.activation(
                                dst,
                                ps[:],
                                mybir.ActivationFunctionType.Identity,
                                bias=bias_col,
                            )
                    nch += 1
                for mt in range(MT):
                    nc.gpsimd.dma_start(
                        out[b, mt * P : (mt + 1) * P, bass.ds(obase, ocols)],
                        otiles[mt][:],
                    )
    # The axon/PJRT exec path serializes nc as-is; finalize here so Bacc's
    # compile passes (register alloc, event-semaphore wait splitting) run.
    nc.finalize()
    return nc


def kernel(pre, W_pre, b_pre):
    global LAST_RESULT
    preh = np.ascontiguousarray(np.asarray(pre, dtype=np.float32).astype(np.float16))
    # Fold the reference's final y+y into the weights/bias: out = (2W)x + 2b.
    # Pre-tile W for a single DMA: wt[p, kt*MT+mt, m] = 2*W[mt*128+m, kt*128+p].
    w2 = (np.asarray(W_pre, dtype=np.float32) * 2.0).astype(np.float16)
    wtil = np.ascontiguousarray(
        w2.reshape(MT, P, KT, P).transpose(3, 2, 0, 1)
    )  # [p, kt, mt, m]
    b2 = np.ascontiguousarray(
        (2.0 * np.asarray(b_pre, dtype=np.float32)).reshape(MT, P).T
    )
    if "nc" not in _cache:
        _cache["nc"] = _build()
    nc = _cache["nc"]
    in_maps = [
        {"pre": preh[i * BPC : (i + 1) * BPC], "wt": wtil, "b2": b2}
        for i in range(NCORES)
    ]
    res = run_bass_kernel_spmd(nc, in_maps, list(range(NCORES)))
    LAST_RESULT = res
    return np.ascontiguousarray(
        np.concatenate([res.results[i]["out"] for i in range(NCORES)], axis=0),
        dtype=np.float32,
    )


# revision 6
# speedup vs baseline: 1.6217x; 1.0187x over previous
"""Trainium2 Bass kernel for y = 2*(einsum('bct,oc->bot', pre, W_pre) + b_pre).

Shapes (hardcoded): pre [16, 512, 4096] f32, W_pre [512, 512] f32, b_pre [512] f32.
Sharding: data-parallel over B across 8 cores (2 batches per core).

Per core: out[b, o, t] = 2*(sum_c W[o,c]*pre[b,c,t] + bias[o]) for 2 batches.
PE matmul computes lhsT.T @ rhs with lhsT = W.T tiles [K=128, M=128] and
rhs = pre tiles [K=128, N=512]; accumulate 4 K-tiles into one PSUM bank,
then ScalarE/DVE apply out = psum + 2*bias on eviction PSUM->SBUF.

All device I/O is fp16 (host casts): the fp32 baseline was HBM-bound
(33.6MB/core at ~358GB/s ~ 94us); fp16 halves traffic to 16.8MB (~47us),
making the kernel PE-bound (~55us of fp16-rate matmul). fp8 would double
the PE rate (DoubleRow) but exceeds the 2e-2 error gate (4e-2 measured
offline on the exact dataset), so fp16 is the precision floor.

DMA instruction count is minimized (HWDGE issue costs ~0.6us each on the
issuing sequencer, serializing the prologue): W is one host-pre-tiled
DMA, each x column-chunk is one DMA covering all 4 K-tiles (transposed
AP), outputs are 12 DMAs. x loads issue on SP (sync), W/bias on ACT
(scalar), outputs on GpSimd (SWDGE) so the three streams don't serialize.
"""

import os
import sys

for _p in ("/opt/trn_rl_repo", "/root/.axon_site/_ro/trn_rl_repo"):
    if os.path.isdir(_p) and _p not in sys.path:
        sys.path.append(_p)

from contextlib import ExitStack

import numpy as np

import concourse.bass as bass
import concourse.tile as tile
from concourse import bacc, mybir
from concourse.bass_utils import run_bass_kernel_spmd

B, C, T = 16, 512, 4096  # batch, channels (in == out), sequence
NCORES = 8
BPC = B // NCORES  # batches per core
P = 128
KT = C // P  # contraction tiles
MT = C // P  # output-channel tiles
NCHUNK = 512  # matmul moving-operand free dim
NCH = T // NCHUNK
# Input DMA column chunks: small first chunk so the first matmul group's
# data lands early, bigger later ones to amortize DMA issue overhead.
XCS = [512, 512, 1024, 2048]
# Output store groups (in NCHUNK units) per batch: taper batch 1 so the
# final DMAs after the last matmul are small.
OGS = {0: [8], 1: [4, 3, 1]}

IN_DT = mybir.dt.float16
OUT_DT = mybir.dt.float16

LAST_RESULT = None  # BassKernelResults of the most recent run (for test harness)
_cache = {}


def _build():
    # Bacc (not plain Bass): its finalize() runs move_matmul_waits_to_ldweights +
    # generate_event_semaphores, which walrus needs.
    nc = bacc.Bacc("TRN2", target_bir_lowering=False, debug=False, num_devices=NCORES)
    # pre viewed as [b, kt, p, t] (same layout as [b, c, t] with c = kt*128+p).
    pre = nc.dram_tensor("pre", [BPC, KT, P, T], IN_DT, kind="ExternalInput").ap()
    # W pre-tiled on host: wt[p, kt*MT+mt, m] = 2*W[mt*128+m, kt*128+p].
    wt = nc.dram_tensor("wt", [P, KT * MT, P], IN_DT, kind="ExternalInput").ap()
    b2 = nc.dram_tensor("b2", [P, MT], mybir.dt.float32, kind="ExternalInput").ap()
    out = nc.dram_tensor("out", [BPC, C, T], OUT_DT, kind="ExternalOutput").ap()

    with ExitStack() as ctx:
        tc = ctx.enter_context(tile.TileContext(nc))
        wpool = ctx.enter_context(tc.tile_pool(name="w", bufs=1))
        bpool = ctx.enter_context(tc.tile_pool(name="bias", bufs=1))
        xpool = ctx.enter_context(tc.tile_pool(name="x", bufs=2))
        opool = ctx.enter_context(tc.tile_pool(name="o", bufs=8))
        pspool = ctx.enter_context(tc.tile_pool(name="ps", bufs=8, space="PSUM"))

        # One DMA per (batch, column chunk), covering all 4 K-tiles: SBUF
        # tile [128, KT, cols] <- dram [kt, p, cols] transposed to [p, kt, cols].
        # Issue order is consumption order; b0 chunk 0 first.
        def load_x(b, xi, off, cols):
            x = xpool.tile([P, KT, cols], IN_DT, name=f"x_{b}_{xi}", tag=f"x{xi}")
            nc.sync.dma_start(
                x[:], pre[b, :, :, bass.ds(off, cols)].transpose([1, 0, 2])
            )
            return x

        xtiles = {}
        xtiles[(0, 0)] = load_x(0, 0, 0, XCS[0])

        # Whole W in one DMA on the other HWDGE engine (ACT): 512KB.
        wtile = wpool.tile([P, KT * MT * P], IN_DT, name="w")
        nc.scalar.dma_start(wtile[:], wt[:].flatten_outer_dims())
        btile = bpool.tile([P, MT], mybir.dt.float32)
        nc.scalar.dma_start(btile[:], b2[:])

        # Warmup matmuls on a memset tile while the first x/W DMAs are in
        # flight: ~4us of sustained PE activity releases the HAM clock gate
        # (1.2 -> 2.4 GHz) before the first real matmul arrives, and they
        # drain before the real data lands so they cost nothing.
        warm = bpool.tile([P, NCHUNK], IN_DT, name="warm")
        nc.vector.memset(warm[:], 0)
        ps_warm = pspool.tile([P, NCHUNK], mybir.dt.float32, tag="ps")
        for _ in range(10):
            nc.tensor.matmul(
                ps_warm[:], warm[:, 0:P], warm[:], start=True, stop=True
            )

        def wslice(kt, mt):
            return wtile[:, (kt * MT + mt) * P : (kt * MT + mt + 1) * P]

        off = XCS[0]
        for xi in range(1, len(XCS)):
            xtiles[(0, xi)] = load_x(0, xi, off, XCS[xi])
            off += XCS[xi]
        off = 0
        for xi in range(len(XCS)):
            xtiles[(1, xi)] = load_x(1, xi, off, XCS[xi])
            off += XCS[xi]

        # nch -> (x tile index, column offset inside that tile)
        xmap = []
        off = 0
        for xi, xcols in enumerate(XCS):
            for o in range(0, xcols, NCHUNK):
                xmap.append((xi, o))
            off += xcols
        assert len(xmap) == NCH

        for b in range(BPC):
            nch = 0
            for og, osize in enumerate(OGS[b]):
                ocols = osize * NCHUNK
                otiles = [
                    opool.tile([P, ocols], OUT_DT, name=f"o_{b}_{og}_{mt}", tag="o")
                    for mt in range(MT)
                ]
                obase = nch * NCHUNK
                for j in range(osize):
                    xi, xoff = xmap[nch]
                    for mt in range(MT):
                        ps = pspool.tile([P, NCHUNK], mybir.dt.float32, tag="ps")
                        for kt in range(KT):
                            nc.tensor.matmul(
                                ps[:],
                                wslice(kt, mt),
                                xtiles[(b, xi)][:, kt, xoff : xoff + NCHUNK],
                                start=(kt == 0),
                                stop=(kt == KT - 1),
                            )
                        # W is pre-scaled by 2 on the host, so only + 2*bias
                        # remains; alternate DVE/ACT so neither engine binds.
                        dst = otiles[mt][:, bass.ts(j, NCHUNK)]
                        bias_col = btile[:, mt : mt + 1]
                        if (j + mt) % 2 == 0:
                            nc.vector.tensor_scalar_add(dst, ps[:], bias_col)
                        else:
                            nc.scalar.activation(
                                dst,
                                ps[:],
                                mybir.ActivationFunctionType.Identity,
                                bias=bias_col,
                            )
                    nch += 1
                for mt in range(MT):
                    nc.scalar.dma_start(
                        out[b, mt * P : (mt + 1) * P, bass.ds(obase, ocols)],
                        otiles[mt][:],
                    )
    # The axon/PJRT exec path serializes nc as-is; finalize here so Bacc's
    # compile passes (register alloc, event-semaphore wait splitting) run.
    nc.finalize()
    return nc


def kernel(pre, W_pre, b_pre):
    global LAST_RESULT
    preh = np.ascontiguousarray(np.asarray(pre, dtype=np.float32).astype(np.float16))
    # Fold the reference's final y+y into the weights/bias: out = (2W)x + 2b.
    # Pre-tile W for a single DMA: wt[p, kt*MT+mt, m] = 2*W[mt*128+m, kt*128+p].
    w2 = (np.asarray(W_pre, dtype=np.float32) * 2.0).astype(np.float16)
    wtil = np.ascontiguousarray(
        w2.reshape(MT, P, KT, P).transpose(3, 2, 0, 1)
    )  # [p, kt, mt, m]
    b2 = np.ascontiguousarray(
        (2.0 * np.asarray(b_pre, dtype=np.float32)).reshape(MT, P).T
    )
    if "nc" not in _cache:
        _cache["nc"] = _build()
    nc = _cache["nc"]
    in_maps = [
        {"pre": preh[i * BPC : (i + 1) * BPC], "wt": wtil, "b2": b2}
        for i in range(NCORES)
    ]
    res = run_bass_kernel_spmd(nc, in_maps, list(range(NCORES)))
    LAST_RESULT = res
    return np.ascontiguousarray(
        np.concatenate([res.results[i]["out"] for i in range(NCORES)], axis=0),
        dtype=np.float32,
    )


# revision 7
# speedup vs baseline: 1.6506x; 1.0179x over previous
"""Trainium2 Bass kernel for y = 2*(einsum('bct,oc->bot', pre, W_pre) + b_pre).

Shapes (hardcoded): pre [16, 512, 4096] f32, W_pre [512, 512] f32, b_pre [512] f32.
Sharding: data-parallel over B across 8 cores (2 batches per core).

Per core: out[b, o, t] = 2*(sum_c W[o,c]*pre[b,c,t] + bias[o]) for 2 batches.
PE matmul computes lhsT.T @ rhs with lhsT = W.T tiles [K=128, M=128] and
rhs = pre tiles [K=128, N<=512]; accumulate 4 K-tiles into one PSUM bank,
then ScalarE/DVE apply out = psum + 2*bias on eviction PSUM->SBUF.

All device I/O is fp16 (host casts): the fp32 baseline was HBM-bound
(33.6MB/core at ~358GB/s ~ 94us); fp16 halves traffic to 16.8MB (~47us),
making the kernel PE-bound (~55us of fp16-rate matmul). fp8 would double
the PE rate (DoubleRow) but exceeds the 2e-2 error gate (4e-2 measured
offline on the exact dataset), so fp16 is the precision floor.

Startup critical path: HWDGE issue costs ~0.6us+ per DMA on the issuing
sequencer and the engines deliver ~250GB/s early on, so the first-matmul
wait is set by (first x chunk + first W half). Both are 256KB: x loads
issue on SP (sync), W (mt-major, 2 half DMAs) + bias on ACT (scalar).
Warmup matmuls on a memset tile run during the DMA wait so the PE HAM
clock gate is already released (2.4GHz) when real matmuls start.
Outputs stream from SBUF on both HWDGE rings, tapered so the last DMAs
after the final matmul are small.
"""

import os
import sys

for _p in ("/opt/trn_rl_repo", "/root/.axon_site/_ro/trn_rl_repo"):
    if os.path.isdir(_p) and _p not in sys.path:
        sys.path.append(_p)

from contextlib import ExitStack

import numpy as np

import concourse.bass as bass
import concourse.tile as tile
from concourse import bacc, mybir
from concourse.bass_utils import run_bass_kernel_spmd

B, C, T = 16, 512, 4096  # batch, channels (in == out), sequence
NCORES = 8
BPC = B // NCORES  # batches per core
P = 128
KT = C // P  # contraction tiles
MT = C // P  # output-channel tiles
NCHUNK = 512  # max matmul moving-operand free dim (PSUM bank limit)
# Input DMA column chunks: small first chunks so the first matmul group's
# data lands early, bigger later ones to amortize DMA issue overhead.
XCS = [256, 256, 512, 1024, 2048]
# Matmul work chunks (cols per PSUM group), derived from XCS boundaries.
WORK = []  # (xi, xoff, ncols)
for _xi, _c in enumerate(XCS):
    for _o in range(0, _c, NCHUNK):
        WORK.append((_xi, _o, min(NCHUNK, _c - _o)))
NCHW = len(WORK)
# Output store groups (in WORK-chunk counts) per batch: taper batch 1 so
# the final DMAs after the last matmul are small.
OGS = {0: [NCHW], 1: [6, 2, 1]}
assert sum(OGS[1]) == NCHW

IN_DT = mybir.dt.float16
OUT_DT = mybir.dt.float16

LAST_RESULT = None  # BassKernelResults of the most recent run (for test harness)
_cache = {}


def _build():
    # Bacc (not plain Bass): its finalize() runs move_matmul_waits_to_ldweights +
    # generate_event_semaphores, which walrus needs.
    nc = bacc.Bacc("TRN2", target_bir_lowering=False, debug=False, num_devices=NCORES)
    # pre viewed as [b, kt, p, t] (same layout as [b, c, t] with c = kt*128+p).
    pre = nc.dram_tensor("pre", [BPC, KT, P, T], IN_DT, kind="ExternalInput").ap()
    # W pre-tiled on host, mt-major: wt[p, mt*KT+kt, m] = 2*W[mt*128+m, kt*128+p]
    # so each half-DMA delivers complete mt groups.
    wt = nc.dram_tensor("wt", [P, KT * MT, P], IN_DT, kind="ExternalInput").ap()
    b2 = nc.dram_tensor("b2", [P, MT], mybir.dt.float32, kind="ExternalInput").ap()
    out = nc.dram_tensor("out", [BPC, C, T], OUT_DT, kind="ExternalOutput").ap()

    with ExitStack() as ctx:
        tc = ctx.enter_context(tile.TileContext(nc))
        wpool = ctx.enter_context(tc.tile_pool(name="w", bufs=1))
        bpool = ctx.enter_context(tc.tile_pool(name="bias", bufs=1))
        xpool = ctx.enter_context(tc.tile_pool(name="x", bufs=2))
        opool = ctx.enter_context(tc.tile_pool(name="o", bufs=8))
        pspool = ctx.enter_context(tc.tile_pool(name="ps", bufs=8, space="PSUM"))

        # One DMA per (batch, column chunk), covering all 4 K-tiles: SBUF
        # tile [128, KT, cols] <- dram [kt, p, cols] transposed to [p, kt, cols].
        # Issue order is consumption order; b0 chunk 0 first.
        def load_x(b, xi, off, cols):
            x = xpool.tile([P, KT, cols], IN_DT, name=f"x_{b}_{xi}", tag=f"x{xi}")
            nc.sync.dma_start(
                x[:], pre[b, :, :, bass.ds(off, cols)].transpose([1, 0, 2])
            )
            return x

        xtiles = {}
        xtiles[(0, 0)] = load_x(0, 0, 0, XCS[0])

        # W in two half DMAs on the other HWDGE engine (ACT), mt-major so the
        # first 256KB covers mt 0-1 completely (the first PSUM groups).
        wtile = wpool.tile([P, KT * MT * P], IN_DT, name="w")
        HW = KT * MT * P // 2
        nc.scalar.dma_start(wtile[:, 0:HW], wt[:, 0 : KT * MT // 2, :])
        nc.scalar.dma_start(wtile[:, HW:], wt[:, KT * MT // 2 :, :])
        btile = bpool.tile([P, MT], mybir.dt.float32)
        nc.scalar.dma_start(btile[:], b2[:])

        def wslice(kt, mt):
            return wtile[:, (mt * KT + kt) * P : (mt * KT + kt + 1) * P]

        # Warmup matmuls on a memset tile while the first x/W DMAs are in
        # flight: ~4us of sustained PE activity releases the HAM clock gate
        # (1.2 -> 2.4 GHz) before the first real matmul arrives, and they
        # drain before the real data lands so they cost nothing.
        warm = bpool.tile([P, NCHUNK], IN_DT, name="warm")
        nc.vector.memset(warm[:], 0)
        ps_warm = pspool.tile([P, NCHUNK], mybir.dt.float32, tag="ps")
        for _ in range(10):
            nc.tensor.matmul(
                ps_warm[:], warm[:, 0:P], warm[:], start=True, stop=True
            )

        off = XCS[0]
        for xi in range(1, len(XCS)):
            xtiles[(0, xi)] = load_x(0, xi, off, XCS[xi])
            off += XCS[xi]
        off = 0
        for xi in range(len(XCS)):
            xtiles[(1, xi)] = load_x(1, xi, off, XCS[xi])
            off += XCS[xi]

        for b in range(BPC):
            nch = 0
            for og, osize in enumerate(OGS[b]):
                chunks = WORK[nch : nch + osize]
                ocols = sum(c[2] for c in chunks)
                obase = sum(c[2] for c in WORK[:nch])
                otiles = [
                    opool.tile([P, ocols], OUT_DT, name=f"o_{b}_{og}_{mt}", tag="o")
                    for mt in range(MT)
                ]
                ooff = 0
                for xi, xoff, ncols in chunks:
                    for mt in range(MT):
                        ps = pspool.tile([P, ncols], mybir.dt.float32, tag="ps")
                        for kt in range(KT):
                            nc.tensor.matmul(
                                ps[:],
                                wslice(kt, mt),
                                xtiles[(b, xi)][:, kt, xoff : xoff + ncols],
                                start=(kt == 0),
                                stop=(kt == KT - 1),
                            )
                        # W is pre-scaled by 2 on the host, so only + 2*bias
                        # remains; alternate DVE/ACT so neither engine binds.
                        dst = otiles[mt][:, ooff : ooff + ncols]
                        bias_col = btile[:, mt : mt + 1]
                        if (nch + mt) % 2 == 0:
                            nc.vector.tensor_scalar_add(dst, ps[:], bias_col)
                        else:
                            nc.scalar.activation(
                                dst,
                                ps[:],
                                mybir.ActivationFunctionType.Identity,
                                bias=bias_col,
                            )
                    ooff += ncols
                    nch += 1
                for mt in range(MT):
                    # Alternate output DMAs across both HWDGE rings so the
                    # tail's issue cost (~0.6us each) is split.
                    eng = nc.scalar if mt % 2 else nc.sync
                    eng.dma_start(
                        out[b, mt * P : (mt + 1) * P, bass.ds(obase, ocols)],
                        otiles[mt][:],
                    )
    # The axon/PJRT exec path serializes nc as-is; finalize here so Bacc's
    # compile passes (register alloc, event-semaphore wait splitting) run.
    nc.finalize()
    return nc


def kernel(pre, W_pre, b_pre):
    global LAST_RESULT
    preh = np.ascontiguousarray(np.asarray(pre, dtype=np.float32).astype(np.float16))
    # Fold the reference's final y+y into the weights/bias: out = (2W)x + 2b.
    # Pre-tile W mt-major: wt[p, mt*KT+kt, m] = 2*W[mt*128+m, kt*128+p].
    w2 = (np.asarray(W_pre, dtype=np.float32) * 2.0).astype(np.float16)
    wtil = np.ascontiguousarray(
        w2.reshape(MT, P, KT, P).transpose(3, 0, 2, 1).reshape(P, KT * MT, P)
    )  # [p, mt, kt, m]
    b2 = np.ascontiguousarray(
        (2.0 * np.asarray(b_pre, dtype=np.float32)).reshape(MT, P).T
    )
    if "nc" not in _cache:
        _cache["nc"] = _build()
    nc = _cache["nc"]
    in_maps = [
        {"pre": preh[i * BPC : (i + 1) * BPC], "wt": wtil, "b2": b2}
        for i in range(NCORES)
    ]
    res = run_bass_kernel_spmd(nc, in_maps, list(range(NCORES)))
    LAST_RESULT = res
    return np.ascontiguousarray(
        np.concatenate([res.results[i]["out"] for i in range(NCORES)], axis=0),
        dtype=np.float32,
    )
